# revision 3
# baseline (speedup 1.0000x reference)
"""BinaryTreeComposer cell on 8 Trainium2 NeuronCores.

Math (per reference):
    g  = lh @ Wl + bl + rh @ Wr + br          # [B, 4D]
    i  = sigmoid(g[:, 0:D]);  lf = sigmoid(g[:, D:2D])
    rf = sigmoid(g[:, 2D:3D]); u = tanh(g[:, 3D:4D])
    c  = i*u + lf*lc + rf*rc;  h = tanh(c)
    return (c, h)

Sharding: column-parallel over the hidden dim D. Core s owns the D/8-wide
column slice [s*256, (s+1)*256) of each of the four gate blocks, i.e. a
[2D=4096, 4*256=1024] slice of the stacked weight matrix [Wl; Wr]. Each core
reads the full (stacked+transposed) activations A = [lh.T; rh.T] and writes
its own [4096, 256] column slice of c and h. Gates are elementwise per
column, so no cross-core communication is needed.

The two GEMMs are fused into a single K=4096 PSUM accumulation. Matmuls run
in float32r (8e11m, rounds via the casting DMA) — full 1 cycle/row PE speed
with ~1e-4 relative error.

Host-side data prep only re-lays-out tensors (transpose + tiling) so every
DMA moves >=16 KiB contiguous lines.
"""

import numpy as np

import concourse.mybir as mybir
import concourse.tile as tile
from concourse import bacc
from concourse.bass_utils import run_bass_kernel_spmd

B = 4096          # batch / node dim
D = 2048          # mem_dim
S = 8             # cores
DC = D // S       # 256: per-core column chunk of D
NG = 4 * DC       # 1024: per-core gate columns (4 gate blocks)
P = 128
KO = (2 * D) // P  # 32 contraction chunks (lh and rh stacked)
MT = B // P        # 32 batch tiles

F32 = mybir.dt.float32
F32R = mybir.dt.float32r


def _build_nc():
    nc = bacc.Bacc("TRN2", target_bir_lowering=False, debug=False, num_devices=S)

    a4 = nc.dram_tensor("a4", [P, MT, KO * P], F32, kind="ExternalInput").ap()
    w4 = nc.dram_tensor("w4", [P, KO, NG], F32, kind="ExternalInput").ap()
    bias = nc.dram_tensor("bias", [P, NG], F32, kind="ExternalInput").ap()
    lc = nc.dram_tensor("lc", [B, DC], F32, kind="ExternalInput").ap()
    rc = nc.dram_tensor("rc", [B, DC], F32, kind="ExternalInput").ap()
    c_out = nc.dram_tensor("c", [B, DC], F32, kind="ExternalOutput").ap()
    h_out = nc.dram_tensor("h", [B, DC], F32, kind="ExternalOutput").ap()

    with tile.TileContext(nc) as tc:
        with (
            tc.tile_pool(name="wpool", bufs=1) as wpool,
            tc.tile_pool(name="apool", bufs=2) as apool,
            tc.tile_pool(name="gpool", bufs=3) as gpool,
            tc.tile_pool(name="gatepool", bufs=6) as gatepool,
            tc.tile_pool(name="cellpool", bufs=4) as cellpool,
            tc.tile_pool(name="tmppool", bufs=4) as tmppool,
            tc.tile_pool(name="outpool", bufs=3) as outpool,
            tc.tile_pool(name="psum", bufs=4, space="PSUM") as psum,
        ):
            # Weights resident in SBUF for the whole kernel (128 KiB/partition),
            # rounded to f32r by the casting DMA.
            w_sb = wpool.tile([P, KO, NG], F32R)
            nc.gpsimd.dma_start(w_sb[:], w4[:])
            bias_sb = wpool.tile([P, NG], F32)
            nc.sync.dma_start(bias_sb[:], bias[:])

            for m in range(MT):
                a_sb = apool.tile([P, KO * P], F32R, tag="a")
                nc.gpsimd.dma_start(a_sb[:], a4[:, m, :])
                a_k = a_sb.rearrange("p (ko bi) -> p ko bi", bi=P)

                g_half = []
                for n in range(2):
                    ps = psum.tile([P, 512], F32, tag="ps")
                    for ko in range(KO):
                        nc.tensor.matmul(
                            ps[:],
                            lhsT=a_k[:, ko, :],
                            rhs=w_sb[:, ko, n * 512:(n + 1) * 512],
                            start=(ko == 0),
                            stop=(ko == KO - 1),
                        )
                    g_sb = gpool.tile([P, 512], F32, tag="g")
                    nc.vector.tensor_add(g_sb[:], ps[:], bias_sb[:, n * 512:(n + 1) * 512])
                    g_half.append(g_sb)

                i_sb = gatepool.tile([P, DC], F32, tag="gate")
                lf_sb = gatepool.tile([P, DC], F32, tag="gate")
                rf_sb = gatepool.tile([P, DC], F32, tag="gate")
                u_sb = gatepool.tile([P, DC], F32, tag="gate")
                Sig = mybir.ActivationFunctionType.Sigmoid
                Tanh = mybir.ActivationFunctionType.Tanh
                nc.scalar.activation(i_sb[:], g_half[0][:, 0:DC], Sig)
                nc.scalar.activation(lf_sb[:], g_half[0][:, DC:2 * DC], Sig)
                nc.scalar.activation(rf_sb[:], g_half[1][:, 0:DC], Sig)
                nc.scalar.activation(u_sb[:], g_half[1][:, DC:2 * DC], Tanh)

                lc_sb = cellpool.tile([P, DC], F32, tag="cin")
                rc_sb = cellpool.tile([P, DC], F32, tag="cin")
                nc.sync.dma_start(lc_sb[:], lc[m * P:(m + 1) * P, :])
                nc.sync.dma_start(rc_sb[:], rc[m * P:(m + 1) * P, :])

                c_sb = outpool.tile([P, DC], F32, tag="c")
                t0 = tmppool.tile([P, DC], F32, tag="t")
                t1 = tmppool.tile([P, DC], F32, tag="t")
                nc.vector.tensor_mul(c_sb[:], i_sb[:], u_sb[:])
                nc.vector.tensor_mul(t0[:], lf_sb[:], lc_sb[:])
                nc.vector.tensor_add(c_sb[:], c_sb[:], t0[:])
                nc.vector.tensor_mul(t1[:], rf_sb[:], rc_sb[:])
                nc.vector.tensor_add(c_sb[:], c_sb[:], t1[:])

                h_sb = outpool.tile([P, DC], F32, tag="h")
                nc.scalar.activation(h_sb[:], c_sb[:], Tanh)

                nc.sync.dma_start(c_out[m * P:(m + 1) * P, :], c_sb[:])
                nc.sync.dma_start(h_out[m * P:(m + 1) * P, :], h_sb[:])

    nc.compile()
    return nc


_NC_CACHE = {}

# Debug knobs (used by the local test harness only; default off).
TRACE = False
TRACE_DIR = None
LAST_RESULT = None


def _get_nc():
    if "nc" not in _NC_CACHE:
        _NC_CACHE["nc"] = _build_nc()
    return _NC_CACHE["nc"]


def kernel(lc, lh, rc, rh, Wl, bl, Wr, br):
    lc = np.ascontiguousarray(lc, dtype=np.float32)
    lh = np.ascontiguousarray(lh, dtype=np.float32)
    rc = np.ascontiguousarray(rc, dtype=np.float32)
    rh = np.ascontiguousarray(rh, dtype=np.float32)
    Wl = np.ascontiguousarray(Wl, dtype=np.float32)
    Wr = np.ascontiguousarray(Wr, dtype=np.float32)
    b = (np.asarray(bl, dtype=np.float32) + np.asarray(br, dtype=np.float32))

    # a4[p, m, ko*P + bi] = A[ko*P + p, m*P + bi] with A = [lh.T; rh.T].
    # For ko < KO/2 rows come from lh, else rh:
    #   lh[b, d] with b=(m bi), d=(ko p) -> [p, m, ko, bi]
    half = KO // 2
    a4 = np.empty((P, MT, KO, P), dtype=np.float32)
    a4[:, :, :half, :] = lh.reshape(MT, P, half, P).transpose(3, 0, 2, 1)
    a4[:, :, half:, :] = rh.reshape(MT, P, half, P).transpose(3, 0, 2, 1)
    a4 = np.ascontiguousarray(a4.reshape(P, MT, KO * P))

    nc = _get_nc()
    in_maps = []
    for s in range(S):
        cols = np.r_[tuple(slice(g * D + s * DC, g * D + (s + 1) * DC) for g in range(4))]
        w_s = np.concatenate([Wl[:, cols], Wr[:, cols]], axis=0)       # [2D, NG]
        w4 = np.ascontiguousarray(w_s.reshape(KO, P, NG).transpose(1, 0, 2))
        bias_s = np.ascontiguousarray(np.broadcast_to(b[cols], (P, NG)))
        in_maps.append({
            "a4": a4,
            "w4": w4,
            "bias": bias_s,
            "lc": np.ascontiguousarray(lc[:, s * DC:(s + 1) * DC]),
            "rc": np.ascontiguousarray(rc[:, s * DC:(s + 1) * DC]),
        })

    kw = {"trace": True, "tmpdir": TRACE_DIR} if TRACE else {}
    res = run_bass_kernel_spmd(nc, in_maps, core_ids=list(range(S)), **kw)
    globals()["LAST_RESULT"] = res
    c_full = np.concatenate([res.results[s]["c"] for s in range(S)], axis=1)
    h_full = np.concatenate([res.results[s]["h"] for s in range(S)], axis=1)
    return (c_full, h_full)


# revision 5
# speedup vs baseline: 1.0135x; 1.0135x over previous
"""BinaryTreeComposer cell on 8 Trainium2 NeuronCores.

Math (per reference):
    g  = lh @ Wl + bl + rh @ Wr + br          # [B, 4D]
    i  = sigmoid(g[:, 0:D]);  lf = sigmoid(g[:, D:2D])
    rf = sigmoid(g[:, 2D:3D]); u = tanh(g[:, 3D:4D])
    c  = i*u + lf*lc + rf*rc;  h = tanh(c)
    return (c, h)

Sharding: column-parallel over the hidden dim D. Core s owns the D/8-wide
column slice [s*256, (s+1)*256) of each of the four gate blocks, i.e. a
[2D=4096, 4*256=1024] slice of the stacked weight matrix [Wl; Wr]. Each core
reads the full (stacked+transposed) activations A = [lh.T; rh.T] and writes
its own [4096, 256] column slice of c and h. Gates are elementwise per
column, so no cross-core communication is needed.

The two GEMMs are fused into a single K=4096 PSUM accumulation. Matmuls run
in float32r (8e11m, rounds via the casting DMA) — full 1 cycle/row PE speed
with ~1e-4 relative error.

Host-side data prep only re-lays-out tensors (transpose + tiling) so every
DMA moves >=16 KiB contiguous lines.
"""

import numpy as np

import concourse.mybir as mybir
import concourse.tile as tile
from concourse import bacc
from concourse.bass_utils import run_bass_kernel_spmd

B = 4096          # batch / node dim
D = 2048          # mem_dim
S = 8             # cores
DC = D // S       # 256: per-core column chunk of D
NG = 4 * DC       # 1024: per-core gate columns (4 gate blocks)
P = 128
KO = (2 * D) // P  # 32 contraction chunks (lh and rh stacked)
MT = B // P        # 32 batch tiles

F32 = mybir.dt.float32
F32R = mybir.dt.float32r


def _build_nc():
    nc = bacc.Bacc("TRN2", target_bir_lowering=False, debug=False, num_devices=S)

    a4 = nc.dram_tensor("a4", [P, MT, KO * P], F32, kind="ExternalInput").ap()
    w4 = nc.dram_tensor("w4", [P, KO, NG], F32, kind="ExternalInput").ap()
    bias = nc.dram_tensor("bias", [P, NG], F32, kind="ExternalInput").ap()
    lc = nc.dram_tensor("lc", [B, DC], F32, kind="ExternalInput").ap()
    rc = nc.dram_tensor("rc", [B, DC], F32, kind="ExternalInput").ap()
    c_out = nc.dram_tensor("c", [B, DC], F32, kind="ExternalOutput").ap()
    h_out = nc.dram_tensor("h", [B, DC], F32, kind="ExternalOutput").ap()

    with tile.TileContext(nc) as tc:
        with (
            tc.tile_pool(name="wpool", bufs=1) as wpool,
            tc.tile_pool(name="apool", bufs=2) as apool,
            tc.tile_pool(name="gpool", bufs=3) as gpool,
            tc.tile_pool(name="gatepool", bufs=6) as gatepool,
            tc.tile_pool(name="cellpool", bufs=4) as cellpool,
            tc.tile_pool(name="tmppool", bufs=4) as tmppool,
            tc.tile_pool(name="outpool", bufs=3) as outpool,
            tc.tile_pool(name="psum", bufs=4, space="PSUM") as psum,
        ):
            # Weights resident in SBUF for the whole kernel (128 KiB/partition),
            # rounded to f32r by the casting DMA. Loaded in 8 chunks so the
            # first matmuls only wait for chunk 0 (~2 MiB) instead of the
            # whole 16 MiB (~47 us at HBM bandwidth).
            WCH = 4  # ko per chunk
            w_tiles = []
            for cidx in range(KO // WCH):
                wt = wpool.tile([P, WCH, NG], F32R, tag=f"w{cidx}")
                nc.gpsimd.dma_start(wt[:], w4[:, cidx * WCH:(cidx + 1) * WCH, :])
                w_tiles.append(wt)
            bias_sb = wpool.tile([P, NG], F32)
            nc.sync.dma_start(bias_sb[:], bias[:])

            for m in range(MT):
                a_sb = apool.tile([P, KO * P], F32R, tag="a")
                nc.gpsimd.dma_start(a_sb[:], a4[:, m, :])
                a_k = a_sb.rearrange("p (ko bi) -> p ko bi", bi=P)

                g_half = []
                for n in range(2):
                    ps = psum.tile([P, 512], F32, tag="ps")
                    for ko in range(KO):
                        nc.tensor.matmul(
                            ps[:],
                            lhsT=a_k[:, ko, :],
                            rhs=w_tiles[ko // WCH][:, ko % WCH, n * 512:(n + 1) * 512],
                            start=(ko == 0),
                            stop=(ko == KO - 1),
                        )
                    g_sb = gpool.tile([P, 512], F32, tag="g")
                    nc.vector.tensor_add(g_sb[:], ps[:], bias_sb[:, n * 512:(n + 1) * 512])
                    g_half.append(g_sb)

                i_sb = gatepool.tile([P, DC], F32, tag="gate")
                lf_sb = gatepool.tile([P, DC], F32, tag="gate")
                rf_sb = gatepool.tile([P, DC], F32, tag="gate")
                u_sb = gatepool.tile([P, DC], F32, tag="gate")
                Sig = mybir.ActivationFunctionType.Sigmoid
                Tanh = mybir.ActivationFunctionType.Tanh
                nc.scalar.activation(i_sb[:], g_half[0][:, 0:DC], Sig)
                nc.scalar.activation(lf_sb[:], g_half[0][:, DC:2 * DC], Sig)
                nc.scalar.activation(rf_sb[:], g_half[1][:, 0:DC], Sig)
                nc.scalar.activation(u_sb[:], g_half[1][:, DC:2 * DC], Tanh)

                lc_sb = cellpool.tile([P, DC], F32, tag="cin")
                rc_sb = cellpool.tile([P, DC], F32, tag="cin")
                nc.sync.dma_start(lc_sb[:], lc[m * P:(m + 1) * P, :])
                nc.sync.dma_start(rc_sb[:], rc[m * P:(m + 1) * P, :])

                c_sb = outpool.tile([P, DC], F32, tag="c")
                t0 = tmppool.tile([P, DC], F32, tag="t")
                t1 = tmppool.tile([P, DC], F32, tag="t")
                nc.vector.tensor_mul(c_sb[:], i_sb[:], u_sb[:])
                nc.vector.tensor_mul(t0[:], lf_sb[:], lc_sb[:])
                nc.vector.tensor_add(c_sb[:], c_sb[:], t0[:])
                nc.vector.tensor_mul(t1[:], rf_sb[:], rc_sb[:])
                nc.vector.tensor_add(c_sb[:], c_sb[:], t1[:])

                h_sb = outpool.tile([P, DC], F32, tag="h")
                nc.scalar.activation(h_sb[:], c_sb[:], Tanh)

                nc.sync.dma_start(c_out[m * P:(m + 1) * P, :], c_sb[:])
                nc.sync.dma_start(h_out[m * P:(m + 1) * P, :], h_sb[:])

    nc.compile()
    return nc


_NC_CACHE = {}

# Debug knobs (used by the local test harness only; default off).
TRACE = False
TRACE_DIR = None
LAST_RESULT = None


def _get_nc():
    if "nc" not in _NC_CACHE:
        _NC_CACHE["nc"] = _build_nc()
    return _NC_CACHE["nc"]


def kernel(lc, lh, rc, rh, Wl, bl, Wr, br):
    lc = np.ascontiguousarray(lc, dtype=np.float32)
    lh = np.ascontiguousarray(lh, dtype=np.float32)
    rc = np.ascontiguousarray(rc, dtype=np.float32)
    rh = np.ascontiguousarray(rh, dtype=np.float32)
    Wl = np.ascontiguousarray(Wl, dtype=np.float32)
    Wr = np.ascontiguousarray(Wr, dtype=np.float32)
    b = (np.asarray(bl, dtype=np.float32) + np.asarray(br, dtype=np.float32))

    # a4[p, m, ko*P + bi] = A[ko*P + p, m*P + bi] with A = [lh.T; rh.T].
    # For ko < KO/2 rows come from lh, else rh:
    #   lh[b, d] with b=(m bi), d=(ko p) -> [p, m, ko, bi]
    half = KO // 2
    a4 = np.empty((P, MT, KO, P), dtype=np.float32)
    a4[:, :, :half, :] = lh.reshape(MT, P, half, P).transpose(3, 0, 2, 1)
    a4[:, :, half:, :] = rh.reshape(MT, P, half, P).transpose(3, 0, 2, 1)
    a4 = np.ascontiguousarray(a4.reshape(P, MT, KO * P))

    nc = _get_nc()
    in_maps = []
    for s in range(S):
        cols = np.r_[tuple(slice(g * D + s * DC, g * D + (s + 1) * DC) for g in range(4))]
        w_s = np.concatenate([Wl[:, cols], Wr[:, cols]], axis=0)       # [2D, NG]
        w4 = np.ascontiguousarray(w_s.reshape(KO, P, NG).transpose(1, 0, 2))
        bias_s = np.ascontiguousarray(np.broadcast_to(b[cols], (P, NG)))
        in_maps.append({
            "a4": a4,
            "w4": w4,
            "bias": bias_s,
            "lc": np.ascontiguousarray(lc[:, s * DC:(s + 1) * DC]),
            "rc": np.ascontiguousarray(rc[:, s * DC:(s + 1) * DC]),
        })

    kw = {"trace": True, "tmpdir": TRACE_DIR} if TRACE else {}
    res = run_bass_kernel_spmd(nc, in_maps, core_ids=list(range(S)), **kw)
    globals()["LAST_RESULT"] = res
    c_full = np.concatenate([res.results[s]["c"] for s in range(S)], axis=1)
    h_full = np.concatenate([res.results[s]["h"] for s in range(S)], axis=1)
    return (c_full, h_full)


# revision 8
# speedup vs baseline: 1.0345x; 1.0207x over previous
"""BinaryTreeComposer cell on 8 Trainium2 NeuronCores.

Math (per reference):
    g  = lh @ Wl + bl + rh @ Wr + br          # [B, 4D]
    i  = sigmoid(g[:, 0:D]);  lf = sigmoid(g[:, D:2D])
    rf = sigmoid(g[:, 2D:3D]); u = tanh(g[:, 3D:4D])
    c  = i*u + lf*lc + rf*rc;  h = tanh(c)
    return (c, h)

Sharding: column-parallel over the hidden dim D. Core s owns the D/8-wide
column slice [s*256, (s+1)*256) of each of the four gate blocks, i.e. a
[2D=4096, 4*256=1024] slice of the stacked weight matrix [Wl; Wr]. Each core
reads the full (stacked+transposed) activations A = [lh.T; rh.T] and writes
its own [4096, 256] column slice of c and h. Gates are elementwise per
column, so no cross-core communication is needed.

The two GEMMs are fused into a single K=4096 PSUM accumulation. Matmuls run
in float32r (8e11m, rounded by the casting DMA) — full 1 cycle/row PE speed
with ~1e-4 relative error.

Weights (16 MiB/core) are streamed in 8 chunks, and the first few batch
tiles' matmuls are emitted in chunk-arrival order so the PE starts working
~10 us in instead of idling for the whole weight load.

Host-side data prep only re-lays-out tensors (transpose + tiling) so every
DMA moves >=16 KiB contiguous lines.
"""

import numpy as np

import concourse.mybir as mybir
import concourse.tile as tile
from concourse import bacc
from concourse.bass_utils import run_bass_kernel_spmd

B = 4096          # batch / node dim
D = 2048          # mem_dim
S = 8             # cores
DC = D // S       # 256: per-core column chunk of D
NG = 4 * DC       # 1024: per-core gate columns (4 gate blocks)
P = 128
KO = (2 * D) // P  # 32 contraction chunks (lh and rh stacked)
MT = B // P        # 32 batch tiles

WCH = 4            # ko per weight chunk
NWC = KO // WCH    # 8 weight chunks
N_PH0 = 3          # batch tiles processed in chunk-arrival order at startup
APOOL_BUFS = 3

F32 = mybir.dt.float32
F32R = mybir.dt.float32r
Sig = mybir.ActivationFunctionType.Sigmoid
Tanh = mybir.ActivationFunctionType.Tanh


def _build_nc():
    nc = bacc.Bacc("TRN2", target_bir_lowering=False, debug=False, num_devices=S)

    a4 = nc.dram_tensor("a4", [P, MT, KO * P], F32, kind="ExternalInput").ap()
    w4 = nc.dram_tensor("w4", [P, KO, NG], F32, kind="ExternalInput").ap()
    bias = nc.dram_tensor("bias", [P, NG], F32, kind="ExternalInput").ap()
    lc = nc.dram_tensor("lc", [B, DC], F32, kind="ExternalInput").ap()
    rc = nc.dram_tensor("rc", [B, DC], F32, kind="ExternalInput").ap()
    c_out = nc.dram_tensor("c", [B, DC], F32, kind="ExternalOutput").ap()
    h_out = nc.dram_tensor("h", [B, DC], F32, kind="ExternalOutput").ap()

    with tile.TileContext(nc) as tc:
        with (
            tc.tile_pool(name="wpool", bufs=1) as wpool,
            tc.tile_pool(name="apool", bufs=APOOL_BUFS) as apool,
            tc.tile_pool(name="gpool", bufs=2) as gpool,
            tc.tile_pool(name="gatepool", bufs=4) as gatepool,
            tc.tile_pool(name="cellpool", bufs=4) as cellpool,
            tc.tile_pool(name="tmppool", bufs=4) as tmppool,
            tc.tile_pool(name="outpool", bufs=2) as outpool,
            tc.tile_pool(name="psum", bufs=8, space="PSUM") as psum,
        ):
            a_tiles = {}
            w_tiles = [None] * NWC

            def load_a(m):
                t = apool.tile([P, KO * P], F32R, tag="a", name=f"a_{m}")
                nc.gpsimd.dma_start(t[:], a4[:, m, :])
                a_tiles[m] = t

            def load_w(cidx):
                wt = wpool.tile([P, WCH, NG], F32R, tag=f"w{cidx}", name=f"w_{cidx}")
                nc.gpsimd.dma_start(wt[:], w4[:, cidx * WCH:(cidx + 1) * WCH, :])
                w_tiles[cidx] = wt

            def mm(m, n, ko):
                nc.tensor.matmul(
                    ps_tiles[(m, n)][:],
                    lhsT=a_tiles[m].rearrange("p (ko bi) -> p ko bi", bi=P)[:, ko, :],
                    rhs=w_tiles[ko // WCH][:, ko % WCH, n * 512:(n + 1) * 512],
                    start=(ko == 0),
                    stop=(ko == KO - 1),
                )

            def epilogue(m):
                g_half = []
                for n in range(2):
                    g_sb = gpool.tile([P, 512], F32, tag="g")
                    nc.vector.tensor_add(
                        g_sb[:], ps_tiles.pop((m, n))[:],
                        bias_sb[:, n * 512:(n + 1) * 512])
                    g_half.append(g_sb)

                i_sb = gatepool.tile([P, DC], F32, tag="gate")
                lf_sb = gatepool.tile([P, DC], F32, tag="gate")
                rf_sb = gatepool.tile([P, DC], F32, tag="gate")
                u_sb = gatepool.tile([P, DC], F32, tag="gate")
                nc.scalar.activation(i_sb[:], g_half[0][:, 0:DC], Sig)
                nc.scalar.activation(lf_sb[:], g_half[0][:, DC:2 * DC], Sig)
                nc.scalar.activation(rf_sb[:], g_half[1][:, 0:DC], Sig)
                nc.scalar.activation(u_sb[:], g_half[1][:, DC:2 * DC], Tanh)

                lc_sb = cellpool.tile([P, DC], F32, tag="cin")
                rc_sb = cellpool.tile([P, DC], F32, tag="cin")
                nc.sync.dma_start(lc_sb[:], lc[m * P:(m + 1) * P, :])
                nc.sync.dma_start(rc_sb[:], rc[m * P:(m + 1) * P, :])

                c_sb = outpool.tile([P, DC], F32, tag="c")
                t0 = tmppool.tile([P, DC], F32, tag="t")
                t1 = tmppool.tile([P, DC], F32, tag="t")
                nc.vector.tensor_mul(c_sb[:], i_sb[:], u_sb[:])
                nc.vector.tensor_mul(t0[:], lf_sb[:], lc_sb[:])
                nc.vector.tensor_add(c_sb[:], c_sb[:], t0[:])
                nc.vector.tensor_mul(t1[:], rf_sb[:], rc_sb[:])
                nc.vector.tensor_add(c_sb[:], c_sb[:], t1[:])

                h_sb = outpool.tile([P, DC], F32, tag="h")
                nc.scalar.activation(h_sb[:], c_sb[:], Tanh)

                nc.sync.dma_start(c_out[m * P:(m + 1) * P, :], c_sb[:])
                nc.sync.dma_start(h_out[m * P:(m + 1) * P, :], h_sb[:])

            # ---- phase 0: stream weights + first N_PH0 a-tiles, matmuls in
            # chunk-arrival order. DMA queue order == `order` (FIFO per engine).
            order = []
            for i in range(max(N_PH0, NWC)):
                if i < N_PH0:
                    order.append(("a", i))
                if i < NWC:
                    order.append(("w", i))

            ps_tiles = {}
            for m in range(N_PH0):
                for n in range(2):
                    ps_tiles[(m, n)] = psum.tile([P, 512], F32, tag="ps",
                                                 name=f"ps_{m}_{n}")

            bias_loaded = False
            next_ko = {(m, n): 0 for m in range(N_PH0) for n in range(2)}
            have_a = set()
            have_w = 0  # chunks 0..have_w-1 arrived
            for kind, idx in order:
                if kind == "a":
                    load_a(idx)
                    have_a.add(idx)
                else:
                    load_w(idx)
                    have_w = idx + 1
                if not bias_loaded:
                    bias_sb = wpool.tile([P, NG], F32)
                    nc.sync.dma_start(bias_sb[:], bias[:])
                    bias_loaded = True
                for m in sorted(have_a):
                    for n in range(2):
                        while next_ko[(m, n)] < have_w * WCH:
                            mm(m, n, next_ko[(m, n)])
                            next_ko[(m, n)] += 1

            for m in range(N_PH0):
                epilogue(m)

            # ---- phase 1: steady-state streaming over remaining batch tiles
            for m in range(N_PH0, MT):
                load_a(m)
                for n in range(2):
                    ps_tiles[(m, n)] = psum.tile([P, 512], F32, tag="ps",
                                                 name=f"ps_{m}_{n}")
                    for ko in range(KO):
                        mm(m, n, ko)
                epilogue(m)

    nc.compile()
    return nc


_NC_CACHE = {}

# Debug knobs (used by the local test harness only; default off).
TRACE = False
TRACE_DIR = None
LAST_RESULT = None


def _get_nc():
    if "nc" not in _NC_CACHE:
        _NC_CACHE["nc"] = _build_nc()
    return _NC_CACHE["nc"]


def kernel(lc, lh, rc, rh, Wl, bl, Wr, br):
    lc = np.ascontiguousarray(lc, dtype=np.float32)
    lh = np.ascontiguousarray(lh, dtype=np.float32)
    rc = np.ascontiguousarray(rc, dtype=np.float32)
    rh = np.ascontiguousarray(rh, dtype=np.float32)
    Wl = np.ascontiguousarray(Wl, dtype=np.float32)
    Wr = np.ascontiguousarray(Wr, dtype=np.float32)
    b = (np.asarray(bl, dtype=np.float32) + np.asarray(br, dtype=np.float32))

    # a4[p, m, ko*P + bi] = A[ko*P + p, m*P + bi] with A = [lh.T; rh.T].
    # For ko < KO/2 rows come from lh, else rh:
    #   lh[b, d] with b=(m bi), d=(ko p) -> [p, m, ko, bi]
    half = KO // 2
    a4 = np.empty((P, MT, KO, P), dtype=np.float32)
    a4[:, :, :half, :] = lh.reshape(MT, P, half, P).transpose(3, 0, 2, 1)
    a4[:, :, half:, :] = rh.reshape(MT, P, half, P).transpose(3, 0, 2, 1)
    a4 = np.ascontiguousarray(a4.reshape(P, MT, KO * P))

    nc = _get_nc()
    in_maps = []
    for s in range(S):
        cols = np.r_[tuple(slice(g * D + s * DC, g * D + (s + 1) * DC) for g in range(4))]
        w_s = np.concatenate([Wl[:, cols], Wr[:, cols]], axis=0)       # [2D, NG]
        w4 = np.ascontiguousarray(w_s.reshape(KO, P, NG).transpose(1, 0, 2))
        bias_s = np.ascontiguousarray(np.broadcast_to(b[cols], (P, NG)))
        in_maps.append({
            "a4": a4,
            "w4": w4,
            "bias": bias_s,
            "lc": np.ascontiguousarray(lc[:, s * DC:(s + 1) * DC]),
            "rc": np.ascontiguousarray(rc[:, s * DC:(s + 1) * DC]),
        })

    kw = {"trace": True, "tmpdir": TRACE_DIR} if TRACE else {}
    res = run_bass_kernel_spmd(nc, in_maps, core_ids=list(range(S)), **kw)
    globals()["LAST_RESULT"] = res
    c_full = np.concatenate([res.results[s]["c"] for s in range(S)], axis=1)
    h_full = np.concatenate([res.results[s]["h"] for s in range(S)], axis=1)
    return (c_full, h_full)


# revision 10
# speedup vs baseline: 1.0745x; 1.0387x over previous
"""BinaryTreeComposer cell on 8 Trainium2 NeuronCores.

Math (per reference):
    g  = lh @ Wl + bl + rh @ Wr + br          # [B, 4D]
    i  = sigmoid(g[:, 0:D]);  lf = sigmoid(g[:, D:2D])
    rf = sigmoid(g[:, 2D:3D]); u = tanh(g[:, 3D:4D])
    c  = i*u + lf*lc + rf*rc;  h = tanh(c)
    return (c, h)

Sharding: column-parallel over the hidden dim D. Core s owns the D/8-wide
column slice [s*256, (s+1)*256) of each of the four gate blocks, i.e. a
[2D=4096, 4*256=1024] slice of the stacked weight matrix [Wl; Wr]. Each core
reads the full (stacked+transposed) activations A = [lh.T; rh.T] and writes
its own [4096, 256] column slice of c and h. Gates are elementwise per
column, so no cross-core communication is needed.

The two GEMMs are fused into a single K=4096 PSUM accumulation. Matmuls run
in float32r (8e11m, rounded by the casting DMA) — full 1 cycle/row PE speed
with ~1e-4 relative error.

Weights (16 MiB/core) are streamed in 16 chunks and activations in 8-ko
subtiles, queued in the order the PE needs them; the first N_PH0 batch
tiles' matmuls are emitted in chunk-arrival order so the PE starts working
as soon as the first ~1.5 MiB lands instead of idling through the whole
weight load.

Host-side data prep only re-lays-out tensors (transpose + tiling) so every
DMA moves >=4 KiB contiguous lines.
"""

import numpy as np

import concourse.mybir as mybir
import concourse.tile as tile
from concourse import bacc
from concourse.bass_utils import run_bass_kernel_spmd

B = 4096          # batch / node dim
D = 2048          # mem_dim
S = 8             # cores
DC = D // S       # 256: per-core column chunk of D
NG = 4 * DC       # 1024: per-core gate columns (4 gate blocks)
P = 128
KO = (2 * D) // P  # 32 contraction chunks (lh and rh stacked)
MT = B // P        # 32 batch tiles

WCH = 2            # ko per weight chunk
NWC = KO // WCH    # 16 weight chunks
ACH = 8            # ko per activation subtile
NAC = KO // ACH    # 4 subtiles per batch tile
N_PH0 = 3          # batch tiles processed in chunk-arrival order at startup
APOOL_BUFS = 13    # a-subtile slots (4 KiB each): 12 resident in phase 0 + rolling

F32 = mybir.dt.float32
F32R = mybir.dt.float32r
Sig = mybir.ActivationFunctionType.Sigmoid
Tanh = mybir.ActivationFunctionType.Tanh


def _build_nc():
    nc = bacc.Bacc("TRN2", target_bir_lowering=False, debug=False, num_devices=S)

    a4 = nc.dram_tensor("a4", [P, MT, KO * P], F32, kind="ExternalInput").ap()
    w4 = nc.dram_tensor("w4", [P, KO, NG], F32, kind="ExternalInput").ap()
    bias = nc.dram_tensor("bias", [P, NG], F32, kind="ExternalInput").ap()
    lc = nc.dram_tensor("lc", [B, DC], F32, kind="ExternalInput").ap()
    rc = nc.dram_tensor("rc", [B, DC], F32, kind="ExternalInput").ap()
    c_out = nc.dram_tensor("c", [B, DC], F32, kind="ExternalOutput").ap()
    h_out = nc.dram_tensor("h", [B, DC], F32, kind="ExternalOutput").ap()

    with tile.TileContext(nc) as tc:
        with (
            tc.tile_pool(name="wpool", bufs=1) as wpool,
            tc.tile_pool(name="apool", bufs=APOOL_BUFS) as apool,
            tc.tile_pool(name="gpool", bufs=2) as gpool,
            tc.tile_pool(name="gatepool", bufs=4) as gatepool,
            tc.tile_pool(name="cellpool", bufs=4) as cellpool,
            tc.tile_pool(name="tmppool", bufs=4) as tmppool,
            tc.tile_pool(name="outpool", bufs=2) as outpool,
            tc.tile_pool(name="psum", bufs=8, space="PSUM") as psum,
        ):
            a_tiles = {}        # (m, sub) -> tile [P, ACH, P]
            w_tiles = [None] * NWC
            ps_tiles = {}

            def load_a(m, sub):
                t = apool.tile([P, ACH, P], F32R, tag="a", name=f"a_{m}_{sub}")
                nc.gpsimd.dma_start(
                    t[:],
                    a4[:, m, sub * ACH * P:(sub + 1) * ACH * P].rearrange(
                        "p (ko bi) -> p ko bi", bi=P))
                a_tiles[(m, sub)] = t

            def load_w(cidx):
                wt = wpool.tile([P, WCH, NG], F32R, tag=f"w{cidx}", name=f"w_{cidx}")
                nc.gpsimd.dma_start(wt[:], w4[:, cidx * WCH:(cidx + 1) * WCH, :])
                w_tiles[cidx] = wt

            def mm(m, n, ko):
                nc.tensor.matmul(
                    ps_tiles[(m, n)][:],
                    lhsT=a_tiles[(m, ko // ACH)][:, ko % ACH, :],
                    rhs=w_tiles[ko // WCH][:, ko % WCH, n * 512:(n + 1) * 512],
                    start=(ko == 0),
                    stop=(ko == KO - 1),
                )

            def epilogue(m):
                g_half = []
                for n in range(2):
                    g_sb = gpool.tile([P, 512], F32, tag="g")
                    nc.vector.tensor_add(
                        g_sb[:], ps_tiles.pop((m, n))[:],
                        bias_sb[:, n * 512:(n + 1) * 512])
                    g_half.append(g_sb)

                i_sb = gatepool.tile([P, DC], F32, tag="gate")
                lf_sb = gatepool.tile([P, DC], F32, tag="gate")
                rf_sb = gatepool.tile([P, DC], F32, tag="gate")
                u_sb = gatepool.tile([P, DC], F32, tag="gate")
                nc.scalar.activation(i_sb[:], g_half[0][:, 0:DC], Sig)
                nc.scalar.activation(lf_sb[:], g_half[0][:, DC:2 * DC], Sig)
                nc.scalar.activation(rf_sb[:], g_half[1][:, 0:DC], Sig)
                nc.scalar.activation(u_sb[:], g_half[1][:, DC:2 * DC], Tanh)

                lc_sb = cellpool.tile([P, DC], F32, tag="cin")
                rc_sb = cellpool.tile([P, DC], F32, tag="cin")
                nc.sync.dma_start(lc_sb[:], lc[m * P:(m + 1) * P, :])
                nc.sync.dma_start(rc_sb[:], rc[m * P:(m + 1) * P, :])

                c_sb = outpool.tile([P, DC], F32, tag="c")
                t0 = tmppool.tile([P, DC], F32, tag="t")
                t1 = tmppool.tile([P, DC], F32, tag="t")
                nc.vector.tensor_mul(c_sb[:], i_sb[:], u_sb[:])
                nc.vector.tensor_mul(t0[:], lf_sb[:], lc_sb[:])
                nc.vector.tensor_add(c_sb[:], c_sb[:], t0[:])
                nc.vector.tensor_mul(t1[:], rf_sb[:], rc_sb[:])
                nc.vector.tensor_add(c_sb[:], c_sb[:], t1[:])

                h_sb = outpool.tile([P, DC], F32, tag="h")
                nc.scalar.activation(h_sb[:], c_sb[:], Tanh)

                nc.sync.dma_start(c_out[m * P:(m + 1) * P, :], c_sb[:])
                nc.sync.dma_start(h_out[m * P:(m + 1) * P, :], h_sb[:])

            # ---- phase 0: stream weights + first N_PH0 batch tiles; DMAs are
            # queued in "first ko that needs them" order and matmuls emitted in
            # arrival order (SWDGE queue drains FIFO per engine).
            events = (
                [("a", (m, s), s * ACH) for m in range(N_PH0) for s in range(NAC)]
                + [("w", c, c * WCH) for c in range(NWC)]
            )
            events.sort(key=lambda e: (e[2], e[0]))  # by need-ko; 'a' before 'w'

            for m in range(N_PH0):
                for n in range(2):
                    ps_tiles[(m, n)] = psum.tile([P, 512], F32, tag="ps",
                                                 name=f"ps_{m}_{n}")

            bias_loaded = False
            next_ko = {(m, n): 0 for m in range(N_PH0) for n in range(2)}
            have_a = {m: 0 for m in range(N_PH0)}  # ko covered per m
            have_w = 0
            for kind, idx, _need in events:
                if kind == "a":
                    m, s = idx
                    load_a(m, s)
                    have_a[m] = (s + 1) * ACH
                else:
                    load_w(idx)
                    have_w = (idx + 1) * WCH
                if not bias_loaded:
                    bias_sb = wpool.tile([P, NG], F32)
                    nc.sync.dma_start(bias_sb[:], bias[:])
                    bias_loaded = True
                for m in range(N_PH0):
                    lim = min(have_w, have_a[m])
                    for n in range(2):
                        while next_ko[(m, n)] < lim:
                            mm(m, n, next_ko[(m, n)])
                            next_ko[(m, n)] += 1

            for m in range(N_PH0):
                epilogue(m)

            # ---- phase 1: steady-state streaming over remaining batch tiles
            for m in range(N_PH0, MT):
                for s in range(NAC):
                    load_a(m, s)
                for n in range(2):
                    ps_tiles[(m, n)] = psum.tile([P, 512], F32, tag="ps",
                                                 name=f"ps_{m}_{n}")
                    for ko in range(KO):
                        mm(m, n, ko)
                epilogue(m)

    nc.compile()
    return nc


_NC_CACHE = {}

# Debug knobs (used by the local test harness only; default off).
TRACE = False
TRACE_DIR = None
LAST_RESULT = None


def _get_nc():
    if "nc" not in _NC_CACHE:
        _NC_CACHE["nc"] = _build_nc()
    return _NC_CACHE["nc"]


def kernel(lc, lh, rc, rh, Wl, bl, Wr, br):
    lc = np.ascontiguousarray(lc, dtype=np.float32)
    lh = np.ascontiguousarray(lh, dtype=np.float32)
    rc = np.ascontiguousarray(rc, dtype=np.float32)
    rh = np.ascontiguousarray(rh, dtype=np.float32)
    Wl = np.ascontiguousarray(Wl, dtype=np.float32)
    Wr = np.ascontiguousarray(Wr, dtype=np.float32)
    b = (np.asarray(bl, dtype=np.float32) + np.asarray(br, dtype=np.float32))

    # a4[p, m, ko*P + bi] = A[ko*P + p, m*P + bi] with A = [lh.T; rh.T].
    # For ko < KO/2 rows come from lh, else rh:
    #   lh[b, d] with b=(m bi), d=(ko p) -> [p, m, ko, bi]
    half = KO // 2
    a4 = np.empty((P, MT, KO, P), dtype=np.float32)
    a4[:, :, :half, :] = lh.reshape(MT, P, half, P).transpose(3, 0, 2, 1)
    a4[:, :, half:, :] = rh.reshape(MT, P, half, P).transpose(3, 0, 2, 1)
    a4 = np.ascontiguousarray(a4.reshape(P, MT, KO * P))

    nc = _get_nc()
    in_maps = []
    for s in range(S):
        cols = np.r_[tuple(slice(g * D + s * DC, g * D + (s + 1) * DC) for g in range(4))]
        w_s = np.concatenate([Wl[:, cols], Wr[:, cols]], axis=0)       # [2D, NG]
        w4 = np.ascontiguousarray(w_s.reshape(KO, P, NG).transpose(1, 0, 2))
        bias_s = np.ascontiguousarray(np.broadcast_to(b[cols], (P, NG)))
        in_maps.append({
            "a4": a4,
            "w4": w4,
            "bias": bias_s,
            "lc": np.ascontiguousarray(lc[:, s * DC:(s + 1) * DC]),
            "rc": np.ascontiguousarray(rc[:, s * DC:(s + 1) * DC]),
        })

    kw = {"trace": True, "tmpdir": TRACE_DIR} if TRACE else {}
    res = run_bass_kernel_spmd(nc, in_maps, core_ids=list(range(S)), **kw)
    globals()["LAST_RESULT"] = res
    c_full = np.concatenate([res.results[s]["c"] for s in range(S)], axis=1)
    h_full = np.concatenate([res.results[s]["h"] for s in range(S)], axis=1)
    return (c_full, h_full)
